# revision 1
# baseline (speedup 1.0000x reference)
"""Trainium2 Bass kernel for the ConditionalDETR sparse-key (topk masking) block.

Computation (per batch image b):
  cls    = outputs_class[b].max(-1)                       # (300,)
  sel    = top-150 of cls (stable, set semantics)         # (300,) 0/1
  boxes  -> pixel xyxy via img_true_sizes[b]
  m[p]   = not (grid point (16i,16j) inside any selected box) | pad[p]   # p = i*32+j
  d[p]   = exclusive prefix sum of m  (destination row for kept tokens)
  out[d[p], b, :] = x[b, :, p]  for m[p]=1 ; remaining rows = 0

Sharding: 8 cores = 4 batches x 2 channel halves (128 ch each); pure data
parallel, identical program on every core (SPMD).

On-device implementation highlights:
  - top-k selection via stable-rank = #{j: cls_j > cls_i} + #{j<i: cls_j == cls_i}
    computed with an all-pairs compare matrix (exact fp32, matches
    jax.lax.top_k tie semantics).
  - point-in-box mask via separable interval masks X^T (q,32) / Y^T (q,32)
    and one PE matmul S = Y^T.T @ X^T (counts; exact small integers).
  - prefix sums via strict-triangular matmuls.
  - permutation applied with one indirect-DMA scatter per tensor; masked-out
    rows get dest=4000 and are dropped by bounds_check (output buffers are
    pre-zeroed by the runner, so dropped rows stay exactly 0).
"""

import sys

import numpy as np

if "/opt/trn_rl_repo" not in sys.path:
    sys.path.insert(0, "/opt/trn_rl_repo")

BS, C, H, W = 4, 256, 32, 32
HW = H * W          # 1024
NQ, NCLS = 300, 80
TOPK = 150
CH = 128            # channels per core
NCORES = 8
CHUNKS = [128, 128, 44]   # 300 queries in partition chunks
NT = HW // 128      # 8 column tiles of x per core

_cache = {}


def _emit(tc, bass, mybir):
    from concourse.masks import make_identity

    nc = tc.nc
    f32 = mybir.dt.float32
    i32 = mybir.dt.int32
    u8 = mybir.dt.uint8
    Alu = mybir.AluOpType
    AX = mybir.AxisListType

    io = _cache["io"]

    with tc.tile_pool(name="sb", bufs=1) as sb, \
         tc.tile_pool(name="ps", bufs=1, space="PSUM") as ps, \
         tc.tile_pool(name="dr", bufs=1, space="DRAM") as dr:

        # ---------------- constants (built on device) ----------------
        ident = sb.tile([128, 128], f32, name="ident")
        make_identity(nc, ident[:])

        g16i = sb.tile([128, 32], i32, name="g16i")
        nc.gpsimd.iota(g16i[:], pattern=[[16, 32]], base=0, channel_multiplier=0)
        g16 = sb.tile([128, 32], f32, name="g16")
        nc.vector.tensor_copy(out=g16[:], in_=g16i[:])

        # T32[a, b] = 1.0 iff a < b  (strict upper triangular)
        T32 = sb.tile([32, 32], f32, name="T32")
        nc.gpsimd.memset(T32[:], 1.0)
        nc.gpsimd.affine_select(
            out=T32[:], in_=T32[:], compare_op=Alu.is_gt, fill=0.0,
            base=0, channel_multiplier=-1, pattern=[[1, 32]])

        # LT[k][p, j] = 1.0 iff j < 128k + p (stable tie-break masks)
        LT = []
        for k, n in enumerate(CHUNKS):
            t = sb.tile([128, NQ], f32, name=f"LT{k}")
            nc.gpsimd.memset(t[:n], 1.0)
            nc.gpsimd.affine_select(
                out=t[:n], in_=t[:n], compare_op=Alu.is_gt, fill=0.0,
                base=128 * k, channel_multiplier=1, pattern=[[-1, NQ]])
            LT.append(t)

        # ---------------- input loads ----------------
        # small latency-critical loads lead the SP HWDGE FIFO; xh follows;
        # ph rides the ACT HWDGE ring so both big loads go in parallel.
        CLS = []
        CRD = []
        for k, n in enumerate(CHUNKS):
            t = sb.tile([128, NCLS], f32, name=f"CLS{k}")
            nc.sync.dma_start(out=t[:n], in_=io["cls"][128 * k:128 * k + n, :])
            CLS.append(t)
            t = sb.tile([128, 4], f32, name=f"CRD{k}")
            nc.sync.dma_start(out=t[:n], in_=io["crd"][128 * k:128 * k + n, :])
            CRD.append(t)

        TSZ = sb.tile([1, 2], i32, name="TSZ")
        nc.sync.dma_start(out=TSZ[:1], in_=io["tsz"])
        PAD8 = sb.tile([32, 32], u8, name="PAD8")
        nc.sync.dma_start(out=PAD8[:32], in_=io["pmask"])

        XH = sb.tile([128, HW], f32, name="XH")
        nc.sync.dma_start(out=XH[:], in_=io["xh"])
        PH = sb.tile([128, HW], f32, name="PH")
        nc.scalar.dma_start(out=PH[:], in_=io["ph"])

        # ---------------- cls max + transpose + broadcast ----------------
        ccol = []
        for k, n in enumerate(CHUNKS):
            t = sb.tile([128, 1], f32, name=f"ccol{k}")
            nc.vector.tensor_reduce(t[:n], CLS[k][:n, :], axis=AX.X, op=Alu.max)
            ccol.append(t)

        CROW = sb.tile([1, NQ], f32, name="CROW")
        for k, n in enumerate(CHUNKS):
            crps = ps.tile([1, 128], f32, tag="crps", bufs=1)
            nc.tensor.transpose(out=crps[:1, :n], in_=ccol[k][:n, :1],
                                identity=ident[:n, :n])
            nc.vector.tensor_copy(out=CROW[:1, 128 * k:128 * k + n],
                                  in_=crps[:1, :n])

        CBC = sb.tile([128, NQ], f32, name="CBC")
        nc.gpsimd.partition_broadcast(CBC[:], CROW[:1, :])

        # img_true_sizes -> f32, broadcast across partitions
        TSF = sb.tile([1, 2], f32, name="TSF")
        nc.vector.tensor_copy(out=TSF[:1], in_=TSZ[:1])
        TSB = sb.tile([128, 2], f32, name="TSB")
        nc.gpsimd.partition_broadcast(TSB[:], TSF[:1, :])

        # ---------------- per-chunk: rank/sel, boxes, X/Y masks, S ----------------
        S32 = ps.tile([32, 32], f32, tag="S32")
        for k, n in enumerate(CHUNKS):
            # stable rank of each query's cls among all 300:
            #   rank = #{j: cls_j > cls_i} + #{j < i: cls_j == cls_i}
            G = sb.tile([128, NQ], f32, tag="G", bufs=2)
            rankG = sb.tile([128, 1], f32, tag="rankG", bufs=2)
            nc.vector.tensor_scalar(out=G[:n], in0=CBC[:n],
                                    scalar1=ccol[k][:n, 0:1], scalar2=None,
                                    op0=Alu.is_gt, op1=Alu.add,
                                    accum_out=rankG[:n])
            E = sb.tile([128, NQ], f32, tag="E", bufs=2)
            rankE = sb.tile([128, 1], f32, tag="rankE", bufs=2)
            nc.vector.scalar_tensor_tensor(
                out=E[:n], in0=CBC[:n], scalar=ccol[k][:n, 0:1], in1=LT[k][:n],
                op0=Alu.is_equal, op1=Alu.mult, accum_out=rankE[:n])
            rank = sb.tile([128, 1], f32, tag="rank", bufs=2)
            nc.vector.tensor_tensor(out=rank[:n], in0=rankG[:n], in1=rankE[:n],
                                    op=Alu.add)
            sel = sb.tile([128, 1], f32, tag="sel", bufs=2)
            nc.vector.tensor_scalar(out=sel[:n], in0=rank[:n],
                                    scalar1=float(TOPK), scalar2=None,
                                    op0=Alu.is_lt)

            # boxes -> scaled xyxy on GPSIMD, concurrent with the DVE rank ops
            # (identical fp32 op order as the reference: sub/add then mult)
            crd = CRD[k]
            w05 = sb.tile([128, 1], f32, tag="w05", bufs=2)
            nc.vector.tensor_scalar(out=w05[:n], in0=crd[:n, 2:3],
                                    scalar1=0.5, scalar2=None, op0=Alu.mult)
            h05 = sb.tile([128, 1], f32, tag="h05", bufs=2)
            nc.vector.tensor_scalar(out=h05[:n], in0=crd[:n, 3:4],
                                    scalar1=0.5, scalar2=None, op0=Alu.mult)

            x1 = sb.tile([128, 1], f32, tag="x1", bufs=2)
            nc.vector.scalar_tensor_tensor(
                out=x1[:n], in0=crd[:n, 0:1], scalar=w05[:n, 0:1],
                in1=TSB[:n, 0:1], op0=Alu.subtract, op1=Alu.mult)
            x2 = sb.tile([128, 1], f32, tag="x2", bufs=2)
            nc.vector.scalar_tensor_tensor(
                out=x2[:n], in0=crd[:n, 0:1], scalar=w05[:n, 0:1],
                in1=TSB[:n, 0:1], op0=Alu.add, op1=Alu.mult)
            y1 = sb.tile([128, 1], f32, tag="y1", bufs=2)
            nc.vector.scalar_tensor_tensor(
                out=y1[:n], in0=crd[:n, 1:2], scalar=h05[:n, 0:1],
                in1=TSB[:n, 1:2], op0=Alu.subtract, op1=Alu.mult)
            y2 = sb.tile([128, 1], f32, tag="y2", bufs=2)
            nc.vector.scalar_tensor_tensor(
                out=y2[:n], in0=crd[:n, 1:2], scalar=h05[:n, 0:1],
                in1=TSB[:n, 1:2], op0=Alu.add, op1=Alu.mult)

            XT = sb.tile([128, 32], f32, tag="XT", bufs=2)
            tmp = sb.tile([128, 32], f32, tag="tmp", bufs=2)
            nc.vector.tensor_scalar(out=tmp[:n], in0=g16[:n],
                                    scalar1=x2[:n, 0:1], scalar2=None,
                                    op0=Alu.is_lt)
            nc.vector.scalar_tensor_tensor(
                out=XT[:n], in0=g16[:n], scalar=x1[:n, 0:1], in1=tmp[:n],
                op0=Alu.is_gt, op1=Alu.mult)
            # fold top-k selection into the X mask (on DVE; waits for sel)
            nc.vector.tensor_tensor(out=XT[:n], in0=XT[:n],
                                    in1=sel[:n, 0:1].to_broadcast([n, 32]),
                                    op=Alu.mult)

            YT = sb.tile([128, 32], f32, tag="YT", bufs=2)
            tmp2 = sb.tile([128, 32], f32, tag="tmp2", bufs=2)
            nc.vector.tensor_scalar(out=tmp2[:n], in0=g16[:n],
                                    scalar1=y2[:n, 0:1], scalar2=None,
                                    op0=Alu.is_lt)
            nc.vector.scalar_tensor_tensor(
                out=YT[:n], in0=g16[:n], scalar=y1[:n, 0:1], in1=tmp2[:n],
                op0=Alu.is_gt, op1=Alu.mult)

            # S[i, j] += sum_q YT[q, i] * XT[q, j]
            nc.tensor.matmul(out=S32[:], lhsT=YT[:n], rhs=XT[:n],
                             start=(k == 0), stop=(k == len(CHUNKS) - 1))

        # ---------------- keep-mask and destination indices ----------------
        PADF = sb.tile([32, 32], f32, name="PADF")
        nc.vector.tensor_copy(out=PADF[:32], in_=PAD8[:32])
        M = sb.tile([32, 32], f32, name="M")
        nc.vector.scalar_tensor_tensor(
            out=M[:32], in0=S32[:], scalar=0.0, in1=PADF[:32],
            op0=Alu.is_equal, op1=Alu.max)

        rsum = sb.tile([32, 1], f32, name="rsum")
        nc.vector.tensor_reduce(rsum[:32], M[:32, :], axis=AX.X, op=Alu.add)
        MT = sb.tile([32, 32], f32, name="MT")
        nc.vector.transpose(MT[:32], M[:32])

        e_ps = ps.tile([32, 32], f32, tag="eps")
        nc.tensor.matmul(out=e_ps[:], lhsT=MT[:32], rhs=T32[:32],
                         start=True, stop=True)
        roff_ps = ps.tile([32, 1], f32, tag="roff")
        nc.tensor.matmul(out=roff_ps[:], lhsT=T32[:32], rhs=rsum[:32],
                         start=True, stop=True)
        roff = sb.tile([32, 1], f32, name="roff")
        nc.vector.tensor_copy(out=roff[:32], in_=roff_ps[:])

        # dest = (e + roff + 4000) - 4000*M : kept rows -> d, dropped -> >= 4000 (OOB)
        A = sb.tile([32, 32], f32, name="A")
        nc.vector.tensor_scalar(out=A[:32], in0=e_ps[:],
                                scalar1=roff[:32, 0:1], scalar2=4000.0,
                                op0=Alu.add, op1=Alu.add)
        DSTF = sb.tile([32, 32], f32, name="DSTF")
        nc.vector.scalar_tensor_tensor(
            out=DSTF[:32], in0=M[:32], scalar=-4000.0, in1=A[:32],
            op0=Alu.mult, op1=Alu.add)
        DI = sb.tile([32, 32], i32, name="DI")
        nc.vector.tensor_copy(out=DI[:32], in_=DSTF[:32])

        # roundtrip through DRAM to regroup (32i, 32j) -> (128p, 8t)
        dest_d = dr.tile([HW], i32, name="dest_d")
        nc.scalar.dma_start(out=dest_d[:].rearrange("(i j) -> i j", j=32),
                            in_=DI[:32])
        DOFF = sb.tile([128, NT], i32, name="DOFF")
        nc.scalar.dma_start(out=DOFF[:],
                            in_=dest_d[:].rearrange("(t p) -> p t", p=128))

        # ------- transpose x / pos into interleaved (token, 2*channel) -------
        # XPT_all[:, 256t:256t+128] = x columns tile t transposed,
        # XPT_all[:, 256t+128:256t+256] = pos columns tile t transposed.
        # Row g of the combined (1024, 256) output = [x_row(g) | pos_row(g)].
        XPT_all = sb.tile([128, 2 * HW], f32, name="XPT_all")
        for t in range(NT):
            cols = slice(128 * t, 128 * (t + 1))
            xp = ps.tile([128, 128], f32, tag="xp", bufs=2)
            nc.tensor.transpose(out=xp[:], in_=XH[:, cols], identity=ident[:])
            nc.vector.tensor_copy(out=XPT_all[:, 256 * t:256 * t + 128], in_=xp[:])
            pp = ps.tile([128, 128], f32, tag="xp", bufs=2)
            nc.tensor.transpose(out=pp[:], in_=PH[:, cols], identity=ident[:])
            nc.vector.tensor_copy(out=XPT_all[:, 256 * t + 128:256 * t + 256],
                                  in_=pp[:])

        # ---------------- scatter rows to their destinations ----------------
        # HW dynamic DMA consumes ONE offset per partition, so issue one
        # indirect DMA per 128-row tile: offsets (128,1), rows of 1 KiB.
        # The 8 scatters write disjoint rows and are FIFO-ordered on the same
        # SWDGE queue; drop the tracker's WAW edges so they pipeline instead
        # of waiting for each other's completion semaphore.
        for t in range(NT):
            nc.gpsimd.indirect_dma_start(
                out=io["skp"],
                out_offset=bass.IndirectOffsetOnAxis(
                    ap=DOFF[:, t:t + 1], axis=0),
                in_=XPT_all[:, 256 * t:256 * (t + 1)],
                in_offset=None,
                bounds_check=HW - 1,
                oob_is_err=False)
            tc.dep_state.clear_tensor_accesses("skp")

        if "dbg" in io:
            nc.sync.dma_start(out=io["dbg_crow"], in_=CROW[:1, :])
            nc.sync.dma_start(out=io["dbg_cbc"], in_=CBC[:, :])
            nc.sync.dma_start(out=io["dbg_m"], in_=M[:32])
            nc.sync.dma_start(out=io["dbg_dstf"], in_=DSTF[:32])
            nc.sync.dma_start(out=io["dbg_doff"], in_=DOFF[:])
            nc.sync.dma_start(out=io["dbg_xt"], in_=XPT_all[:, :HW])


def _build(dbg=False):
    if "nc" in _cache:
        return _cache["nc"]
    from concourse import bacc, mybir, tile
    import concourse.bass as bass

    dt = mybir.dt
    nc = bacc.Bacc("TRN2", target_bir_lowering=False, debug=False,
                   enable_asserts=False, num_devices=NCORES)

    io = {
        "xh": nc.dram_tensor("xh", [CH, HW], dt.float32, kind="ExternalInput").ap(),
        "ph": nc.dram_tensor("ph", [CH, HW], dt.float32, kind="ExternalInput").ap(),
        "cls": nc.dram_tensor("cls", [NQ, NCLS], dt.float32, kind="ExternalInput").ap(),
        "crd": nc.dram_tensor("crd", [NQ, 4], dt.float32, kind="ExternalInput").ap(),
        "tsz": nc.dram_tensor("tsz", [1, 2], dt.int32, kind="ExternalInput").ap(),
        "pmask": nc.dram_tensor("pmask", [H, W], dt.uint8, kind="ExternalInput").ap(),
        "skp": nc.dram_tensor("skp", [HW, 2 * CH], dt.float32,
                              kind="ExternalOutput").ap(),
    }
    if dbg:
        io["dbg"] = True
        io["dbg_crow"] = nc.dram_tensor("dbg_crow", [1, NQ], dt.float32, kind="ExternalOutput").ap()
        io["dbg_cbc"] = nc.dram_tensor("dbg_cbc", [128, NQ], dt.float32, kind="ExternalOutput").ap()
        io["dbg_m"] = nc.dram_tensor("dbg_m", [32, 32], dt.float32, kind="ExternalOutput").ap()
        io["dbg_dstf"] = nc.dram_tensor("dbg_dstf", [32, 32], dt.float32, kind="ExternalOutput").ap()
        io["dbg_doff"] = nc.dram_tensor("dbg_doff", [128, NT], dt.int32, kind="ExternalOutput").ap()
        io["dbg_xt"] = nc.dram_tensor("dbg_xt", [128, HW], dt.float32, kind="ExternalOutput").ap()
    _cache["io"] = io

    with tile.TileContext(nc) as tc:
        _emit(tc, bass, mybir)
    nc.compile()
    _cache["nc"] = nc
    return nc


def _in_maps(x, pos_embed, mask_u8, outputs_coord, outputs_class, its):
    maps = []
    for core in range(NCORES):
        b, h = divmod(core, 2)
        maps.append({
            "xh": np.ascontiguousarray(x[b].reshape(C, HW)[h * CH:(h + 1) * CH]),
            "ph": np.ascontiguousarray(
                pos_embed[b].reshape(C, HW)[h * CH:(h + 1) * CH]),
            "cls": np.ascontiguousarray(outputs_class[b]),
            "crd": np.ascontiguousarray(outputs_coord[b]),
            "tsz": np.ascontiguousarray(its[b:b + 1]),
            "pmask": np.ascontiguousarray(mask_u8[b]),
        })
    return maps


def kernel(x, pos_embed, mask, outputs_coord, outputs_class,
           img_true_sizes, batched_h, batched_w, _trace=False):
    assert int(batched_h) == 512 and int(batched_w) == 512

    x = np.asarray(x, dtype=np.float32)
    pos_embed = np.asarray(pos_embed, dtype=np.float32)
    mask_u8 = np.asarray(mask).astype(np.uint8)
    outputs_coord = np.asarray(outputs_coord, dtype=np.float32)
    outputs_class = np.asarray(outputs_class, dtype=np.float32)
    its = np.asarray(img_true_sizes, dtype=np.int32)

    nc = _build()
    from concourse import bass_utils
    res = bass_utils.run_bass_kernel_spmd(
        nc, _in_maps(x, pos_embed, mask_u8, outputs_coord, outputs_class, its),
        core_ids=list(range(NCORES)), trace=_trace)

    sk = np.empty((HW, BS, C), np.float32)
    sp = np.empty((HW, BS, C), np.float32)
    for core in range(NCORES):
        b, h = divmod(core, 2)
        skp = res.results[core]["skp"]
        sk[:, b, h * CH:(h + 1) * CH] = skp[:, :CH]
        sp[:, b, h * CH:(h + 1) * CH] = skp[:, CH:]
    if _trace:
        kernel.last_results = res
    return sk, sp



# revision 5
# speedup vs baseline: 2.0523x; 2.0523x over previous
"""Trainium2 Bass kernel for the ConditionalDETR sparse-key (topk masking) block.

Computation (per batch image b):
  cls    = outputs_class[b].max(-1)                       # (300,)
  sel    = top-150 of cls (stable, set semantics)         # (300,) 0/1
  boxes  -> pixel xyxy via img_true_sizes[b]
  m[p]   = not (grid point (16i,16j) inside any selected box) | pad[p]   # p = i*32+j
  d[p]   = exclusive prefix sum of m  (destination row for kept tokens)
  out[d[p], b, :] = x[b, :, p]  for m[p]=1 ; remaining rows = 0

Sharding: 8 cores = 4 batches x 2 channel halves (128 ch each); pure data
parallel, identical program on every core (SPMD).

On-device implementation highlights:
  - all small inputs (cls padded to 384 rows with -1e30, crd, true sizes
    replicated per partition, padding mask as f32) ride ONE host-packed
    [128, 288] staging tensor -> one DMA.
  - top-k selection via stable-rank = #{j: cls_j > cls_i} + #{j<i: cls_j == cls_i}
    (exact fp32, matches jax.lax.top_k tie semantics).
  - point-in-box mask via separable interval masks X^T/Y^T and one PE matmul
    S = Y^T.T @ X^T (counts; exact small integers).
  - prefix sums via strict-triangular matmuls; destination indices are
    converted to the int16 [16, 64] wrapped layout fully in SBUF (transpose +
    strided convert-copies), no DRAM roundtrip.
  - the permutation is applied with ONE dma_scatter_add (1024 indices, 1 KiB
    rows) into a [2049, 256] output window: kept tokens add onto the
    runner-pre-zeroed rows 0..1023 (add == write), dropped tokens land in the
    trash region rows 1024..2048 which the host slices off.
"""

import sys

import numpy as np

if "/opt/trn_rl_repo" not in sys.path:
    sys.path.insert(0, "/opt/trn_rl_repo")

BS, C, H, W = 4, 256, 32, 32
HW = H * W          # 1024
NQ, NCLS = 300, 80
NQP = 384           # queries padded to 3x128
TOPK = 150
CH = 128            # channels per core
NCORES = 8
NCHUNK = 3
NT = HW // 128      # 8 column tiles of x per core
NROW_EXT = 2 * HW + 1   # scatter window: rows >= HW are trash

SM_W = 288          # smalls staging width
O_CLS, O_CRD, O_TSZ, O_PAD = 0, 240, 252, 254

_cache = {}


def _emit(tc, bass, mybir):
    from concourse.masks import make_identity

    nc = tc.nc
    f32 = mybir.dt.float32
    i16 = mybir.dt.int16
    Alu = mybir.AluOpType
    AX = mybir.AxisListType

    io = _cache["io"]

    with tc.tile_pool(name="sb", bufs=1) as sb, \
         tc.tile_pool(name="ps", bufs=1, space="PSUM") as ps:

        # ---------------- input loads ----------------
        # smalls first (unblocks the cls/box chain), then the two big loads on
        # separate HWDGE rings so their transfers pipeline on the DMA engines.
        SM = sb.tile([128, SM_W], f32, name="SM")
        nc.sync.dma_start(out=SM[:], in_=io["sm"])
        XPH = sb.tile([128, 2 * HW], f32, name="XPH")
        nc.sync.dma_start(out=XPH[:, :HW], in_=io["xh"])
        nc.scalar.dma_start(out=XPH[:, HW:], in_=io["ph"])

        # ---------------- constants (built on device) ----------------
        ident = sb.tile([128, 128], f32, name="ident")
        make_identity(nc, ident[:])

        g16i = sb.tile([128, 32], mybir.dt.int32, name="g16i")
        nc.gpsimd.iota(g16i[:], pattern=[[16, 32]], base=0, channel_multiplier=0)
        g16 = sb.tile([128, 32], f32, name="g16")
        nc.vector.tensor_copy(out=g16[:], in_=g16i[:])

        # T32[a, b] = 1.0 iff a < b  (strict upper triangular)
        T32 = sb.tile([32, 32], f32, name="T32")
        nc.gpsimd.memset(T32[:], 1.0)
        nc.gpsimd.affine_select(
            out=T32[:], in_=T32[:], compare_op=Alu.is_gt, fill=0.0,
            base=0, channel_multiplier=-1, pattern=[[1, 32]])

        # LT[k][p, j] = 1.0 iff j < 128k + p (stable tie-break masks)
        LT = []
        for k in range(NCHUNK):
            t = sb.tile([128, NQP], f32, name=f"LT{k}")
            nc.gpsimd.memset(t[:], 1.0)
            nc.gpsimd.affine_select(
                out=t[:], in_=t[:], compare_op=Alu.is_gt, fill=0.0,
                base=128 * k, channel_multiplier=1, pattern=[[-1, NQP]])
            LT.append(t)

        # ---------------- cls max + transpose + broadcast ----------------
        ccol = sb.tile([128, NCHUNK], f32, name="ccol")
        nc.vector.tensor_reduce(
            ccol[:], SM[:, O_CLS:O_CLS + NCHUNK * NCLS].rearrange(
                "p (k c) -> p k c", c=NCLS),
            axis=AX.X, op=Alu.max)

        CROW = sb.tile([1, NQP], f32, name="CROW")
        for k in range(NCHUNK):
            crps = ps.tile([1, 128], f32, tag="crps", bufs=1)
            nc.tensor.transpose(out=crps[:1, :], in_=ccol[:, k:k + 1],
                                identity=ident[:])
            nc.scalar.copy(out=CROW[:1, 128 * k:128 * (k + 1)],
                           in_=crps[:1, :])
        CBC = sb.tile([128, NQP], f32, name="CBC")
        nc.gpsimd.partition_broadcast(CBC[:], CROW[:1, :])

        # ---------------- boxes -> scaled xyxy, batched over chunks ----------
        # CRDR[:, 3c + k] = crd[128k + p, c]  (c-major for per-chunk scalars)
        CRDR = sb.tile([128, 12], f32, name="CRDR")
        nc.vector.tensor_copy(
            out=CRDR[:].rearrange("p (c k) -> p k c", k=NCHUNK),
            in_=SM[:, O_CRD:O_CRD + 12].rearrange("p (k c) -> p k c", c=4))
        cx, cy = CRDR[:, 0:3], CRDR[:, 3:6]
        bw, bh = CRDR[:, 6:9], CRDR[:, 9:12]

        w05 = sb.tile([128, 3], f32, name="w05")
        nc.vector.tensor_scalar(out=w05[:], in0=bw, scalar1=0.5, scalar2=None,
                                op0=Alu.mult)
        h05 = sb.tile([128, 3], f32, name="h05")
        nc.vector.tensor_scalar(out=h05[:], in0=bh, scalar1=0.5, scalar2=None,
                                op0=Alu.mult)
        xm = sb.tile([128, 3], f32, name="xm")
        nc.vector.tensor_tensor(out=xm[:], in0=cx, in1=w05[:], op=Alu.subtract)
        xp = sb.tile([128, 3], f32, name="xp")
        nc.vector.tensor_tensor(out=xp[:], in0=cx, in1=w05[:], op=Alu.add)
        ym = sb.tile([128, 3], f32, name="ym")
        nc.vector.tensor_tensor(out=ym[:], in0=cy, in1=h05[:], op=Alu.subtract)
        yp = sb.tile([128, 3], f32, name="yp")
        nc.vector.tensor_tensor(out=yp[:], in0=cy, in1=h05[:], op=Alu.add)
        x1 = sb.tile([128, 3], f32, name="x1")
        nc.vector.tensor_scalar(out=x1[:], in0=xm[:],
                                scalar1=SM[:, O_TSZ:O_TSZ + 1], scalar2=None,
                                op0=Alu.mult)
        x2 = sb.tile([128, 3], f32, name="x2")
        nc.vector.tensor_scalar(out=x2[:], in0=xp[:],
                                scalar1=SM[:, O_TSZ:O_TSZ + 1], scalar2=None,
                                op0=Alu.mult)
        y1 = sb.tile([128, 3], f32, name="y1")
        nc.vector.tensor_scalar(out=y1[:], in0=ym[:],
                                scalar1=SM[:, O_TSZ + 1:O_TSZ + 2], scalar2=None,
                                op0=Alu.mult)
        y2 = sb.tile([128, 3], f32, name="y2")
        nc.vector.tensor_scalar(out=y2[:], in0=yp[:],
                                scalar1=SM[:, O_TSZ + 1:O_TSZ + 2], scalar2=None,
                                op0=Alu.mult)

        # ---------------- per-chunk: rank/sel, X/Y masks, S ----------------
        S32 = ps.tile([32, 32], f32, tag="S32")
        for k in range(NCHUNK):
            cck = ccol[:, k:k + 1]
            G = sb.tile([128, NQP], f32, tag="G", bufs=2)
            rankG = sb.tile([128, 1], f32, tag="rankG", bufs=2)
            nc.vector.tensor_scalar(out=G[:], in0=CBC[:],
                                    scalar1=cck, scalar2=None,
                                    op0=Alu.is_gt, op1=Alu.add,
                                    accum_out=rankG[:])
            E = sb.tile([128, NQP], f32, tag="E", bufs=2)
            rankE = sb.tile([128, 1], f32, tag="rankE", bufs=2)
            nc.vector.scalar_tensor_tensor(
                out=E[:], in0=CBC[:], scalar=cck, in1=LT[k][:],
                op0=Alu.is_equal, op1=Alu.mult, accum_out=rankE[:])
            rank = sb.tile([128, 1], f32, tag="rank", bufs=2)
            nc.vector.tensor_tensor(out=rank[:], in0=rankG[:], in1=rankE[:],
                                    op=Alu.add)
            sel = sb.tile([128, 1], f32, tag="sel", bufs=2)
            nc.vector.tensor_scalar(out=sel[:], in0=rank[:],
                                    scalar1=float(TOPK), scalar2=None,
                                    op0=Alu.is_lt)

            XT = sb.tile([128, 32], f32, tag="XT", bufs=2)
            tmp = sb.tile([128, 32], f32, tag="tmp", bufs=2)
            nc.vector.tensor_scalar(out=tmp[:], in0=g16[:],
                                    scalar1=x2[:, k:k + 1], scalar2=None,
                                    op0=Alu.is_lt)
            nc.vector.scalar_tensor_tensor(
                out=XT[:], in0=g16[:], scalar=x1[:, k:k + 1], in1=tmp[:],
                op0=Alu.is_gt, op1=Alu.mult)
            nc.vector.tensor_tensor(out=XT[:], in0=XT[:],
                                    in1=sel[:, 0:1].to_broadcast([128, 32]),
                                    op=Alu.mult)

            YT = sb.tile([128, 32], f32, tag="YT", bufs=2)
            tmp2 = sb.tile([128, 32], f32, tag="tmp2", bufs=2)
            nc.vector.tensor_scalar(out=tmp2[:], in0=g16[:],
                                    scalar1=y2[:, k:k + 1], scalar2=None,
                                    op0=Alu.is_lt)
            nc.vector.scalar_tensor_tensor(
                out=YT[:], in0=g16[:], scalar=y1[:, k:k + 1], in1=tmp2[:],
                op0=Alu.is_gt, op1=Alu.mult)

            # S[i, j] += sum_q YT[q, i] * XT[q, j]
            nc.tensor.matmul(out=S32[:], lhsT=YT[:], rhs=XT[:],
                             start=(k == 0), stop=(k == NCHUNK - 1))

        # ---------------- keep-mask and destination indices ----------------
        M = sb.tile([32, 32], f32, name="M")
        nc.vector.scalar_tensor_tensor(
            out=M[:32], in0=S32[:], scalar=0.0, in1=SM[0:32, O_PAD:O_PAD + 32],
            op0=Alu.is_equal, op1=Alu.max)

        rsum = sb.tile([32, 1], f32, name="rsum")
        nc.vector.tensor_reduce(rsum[:32], M[:32, :], axis=AX.X, op=Alu.add)
        MT = sb.tile([32, 32], f32, name="MT")
        nc.vector.transpose(MT[:32], M[:32])

        e_ps = ps.tile([32, 32], f32, tag="eps")
        nc.tensor.matmul(out=e_ps[:], lhsT=MT[:32], rhs=T32[:32],
                         start=True, stop=True)
        roff_ps = ps.tile([32, 1], f32, tag="roff")
        nc.tensor.matmul(out=roff_ps[:], lhsT=T32[:32], rhs=rsum[:32],
                         start=True, stop=True)
        roff = sb.tile([32, 1], f32, name="roff")
        nc.vector.tensor_copy(out=roff[:32], in_=roff_ps[:])

        # dest = (e + roff + 1024) - 1024*M : kept -> d, dropped -> trash rows
        A = sb.tile([32, 32], f32, name="A")
        nc.vector.tensor_scalar(out=A[:32], in0=e_ps[:],
                                scalar1=roff[:32, 0:1], scalar2=float(HW),
                                op0=Alu.add, op1=Alu.add)
        DSTF = sb.tile([32, 32], f32, name="DSTF")
        nc.vector.scalar_tensor_tensor(
            out=DSTF[:32], in0=M[:32], scalar=-float(HW), in1=A[:32],
            op0=Alu.mult, op1=Alu.add)
        # int16 idx layout: IDX16[16s + q, c] = dest(token 16c + q), wrapped in
        # 16 partitions and replicated into all 8 stripes s.
        # c = 2a + b  ->  token 32a + 16b + q  ->  DSTF[a, 16b + q].
        # Stay quadrant-aligned: transpose DSTF's column halves on PE
        # (partitions 0..16), interleave into IDXF, then replicate 16 -> 128
        # partitions with a block-identity matmul.
        DTps = ps.tile([16, 64], f32, tag="dt")
        nc.tensor.transpose(out=DTps[:, 0:32], in_=DSTF[:32, 0:16],
                            identity=ident[:32, :32])
        nc.tensor.transpose(out=DTps[:, 32:64], in_=DSTF[:32, 16:32],
                            identity=ident[:32, :32])
        IDXF = sb.tile([16, HW // 16], f32, name="IDXF")
        ev = IDXF[:, :].rearrange("p (a b) -> p b a", b=2)
        nc.vector.tensor_copy(out=ev[:, 0:1, :], in_=DTps[:, 0:32])
        nc.vector.tensor_copy(out=ev[:, 1:2, :], in_=DTps[:, 32:64])

        REP = sb.tile([16, 128], f32, name="REP")
        for s in range(8):
            nc.scalar.copy(out=REP[:, 16 * s:16 * (s + 1)],
                           in_=ident[0:16, 0:16])
        IDXP = ps.tile([128, HW // 16], f32, tag="idxp")
        nc.tensor.matmul(out=IDXP[:], lhsT=REP[:], rhs=IDXF[:],
                         start=True, stop=True)
        IDX16 = sb.tile([128, HW // 16], i16, name="IDX16")
        nc.vector.tensor_copy(out=IDX16[:], in_=IDXP[:])

        # ------- transpose x / pos into interleaved (token, 2*channel) -------
        # XPT_all[p, 256t + c]     = x[c, 128t + p]
        # XPT_all[p, 256t + 128+c] = pos[c, 128t + p]
        XPT_all = sb.tile([128, 2 * HW], f32, name="XPT_all")
        for t in range(NT):
            xp_ = ps.tile([128, 128], f32, tag="xp", bufs=2)
            nc.tensor.transpose(out=xp_[:], in_=XPH[:, 128 * t:128 * (t + 1)],
                                identity=ident[:])
            nc.vector.tensor_copy(out=XPT_all[:, 256 * t:256 * t + 128],
                                  in_=xp_[:])
            pp = ps.tile([128, 128], f32, tag="xp", bufs=2)
            nc.tensor.transpose(out=pp[:],
                                in_=XPH[:, HW + 128 * t:HW + 128 * (t + 1)],
                                identity=ident[:])
            nc.scalar.copy(out=XPT_all[:, 256 * t + 128:256 * (t + 1)],
                           in_=pp[:])

        # ---------------- one scatter for the whole permutation ----------------
        nc.gpsimd.dma_scatter_add(
            out_ap=io["skp"],
            in_ap=XPT_all[:].rearrange("p (j e) -> p j e", e=2 * CH),
            idxs_ap=IDX16[:],
            num_idxs=HW,
            num_idxs_reg=HW,
            elem_size=2 * CH,
        )

        if "dbg" in io:
            nc.sync.dma_start(out=io["dbg_m"], in_=M[:32])
            nc.sync.dma_start(out=io["dbg_dstf"], in_=DSTF[:32])
            nc.sync.dma_start(out=io["dbg_idx"], in_=IDX16[:])
            nc.sync.dma_start(out=io["dbg_xt"], in_=XPT_all[:, :HW])


def _build(dbg=False):
    if "nc" in _cache:
        return _cache["nc"]
    from concourse import bacc, mybir, tile
    import concourse.bass as bass

    dt = mybir.dt
    nc = bacc.Bacc("TRN2", target_bir_lowering=False, debug=False,
                   enable_asserts=False, num_devices=NCORES)

    io = {
        "xh": nc.dram_tensor("xh", [CH, HW], dt.float32, kind="ExternalInput").ap(),
        "ph": nc.dram_tensor("ph", [CH, HW], dt.float32, kind="ExternalInput").ap(),
        "sm": nc.dram_tensor("sm", [128, SM_W], dt.float32, kind="ExternalInput").ap(),
        "skp": nc.dram_tensor("skp", [NROW_EXT, 2 * CH], dt.float32,
                              kind="ExternalOutput").ap(),
    }
    if dbg:
        io["dbg"] = True
        io["dbg_m"] = nc.dram_tensor("dbg_m", [32, 32], dt.float32, kind="ExternalOutput").ap()
        io["dbg_dstf"] = nc.dram_tensor("dbg_dstf", [32, 32], dt.float32, kind="ExternalOutput").ap()
        io["dbg_idx"] = nc.dram_tensor("dbg_idx", [128, HW // 16], dt.int16, kind="ExternalOutput").ap()
        io["dbg_xt"] = nc.dram_tensor("dbg_xt", [128, HW], dt.float32, kind="ExternalOutput").ap()
    _cache["io"] = io

    with tile.TileContext(nc) as tc:
        _emit(tc, bass, mybir)
    nc.compile()
    _cache["nc"] = nc
    return nc


def _smalls(cls_b, crd_b, ts_b, mask_b):
    sm = np.zeros((128, SM_W), np.float32)
    clsp = np.full((NQP, NCLS), -1e30, np.float32)
    clsp[:NQ] = cls_b
    sm[:, O_CLS:O_CLS + NCHUNK * NCLS] = (
        clsp.reshape(NCHUNK, 128, NCLS).transpose(1, 0, 2).reshape(128, -1))
    crdp = np.zeros((NQP, 4), np.float32)
    crdp[:NQ] = crd_b
    sm[:, O_CRD:O_CRD + NCHUNK * 4] = (
        crdp.reshape(NCHUNK, 128, 4).transpose(1, 0, 2).reshape(128, -1))
    sm[:, O_TSZ] = float(ts_b[0])
    sm[:, O_TSZ + 1] = float(ts_b[1])
    sm[0:32, O_PAD:O_PAD + 32] = mask_b.astype(np.float32)
    return sm


def _in_maps(x, pos_embed, mask_u8, outputs_coord, outputs_class, its):
    maps = []
    for core in range(NCORES):
        b, h = divmod(core, 2)
        maps.append({
            "xh": np.ascontiguousarray(x[b].reshape(C, HW)[h * CH:(h + 1) * CH]),
            "ph": np.ascontiguousarray(
                pos_embed[b].reshape(C, HW)[h * CH:(h + 1) * CH]),
            "sm": _smalls(outputs_class[b], outputs_coord[b], its[b],
                          mask_u8[b]),
        })
    return maps


def kernel(x, pos_embed, mask, outputs_coord, outputs_class,
           img_true_sizes, batched_h, batched_w, _trace=False):
    assert int(batched_h) == 512 and int(batched_w) == 512

    x = np.asarray(x, dtype=np.float32)
    pos_embed = np.asarray(pos_embed, dtype=np.float32)
    mask_u8 = np.asarray(mask).astype(np.uint8)
    outputs_coord = np.asarray(outputs_coord, dtype=np.float32)
    outputs_class = np.asarray(outputs_class, dtype=np.float32)
    its = np.asarray(img_true_sizes, dtype=np.int32)

    nc = _build()
    from concourse import bass_utils
    res = bass_utils.run_bass_kernel_spmd(
        nc, _in_maps(x, pos_embed, mask_u8, outputs_coord, outputs_class, its),
        core_ids=list(range(NCORES)), trace=_trace)

    sk = np.empty((HW, BS, C), np.float32)
    sp = np.empty((HW, BS, C), np.float32)
    for core in range(NCORES):
        b, h = divmod(core, 2)
        skp = res.results[core]["skp"]
        sk[:, b, h * CH:(h + 1) * CH] = skp[:HW, :CH]
        sp[:, b, h * CH:(h + 1) * CH] = skp[:HW, CH:]
    if _trace:
        kernel.last_results = res
    return sk, sp
